# revision 32
# baseline (speedup 1.0000x reference)
"""DeepseekV3-style MoE block on 8 Trainium2 NeuronCores (expert-parallel).

Sharding strategy (v2, bf16 data path):
  - The 64 routed experts are sharded 8-per-core.  Expert columns are
    *rotated* per core so every core's own experts sit at columns 0..7 of its
    (rotated) router output; group-limited routing is invariant under group
    rotation because groups are scored independently.
  - The router (gate matmul + sigmoid + group top-k) is replicated, fp32.
  - The shared expert is TP-sharded on its intermediate dim (96 rows/core).
  - Each core computes a partial dense [T, H] fp32 accumulator (shared slice
    + its 8 experts' combine-weighted FFN outputs scattered back by token
    id), then a ReduceScatter(add) sums across cores; the host concatenates
    the 8 shards.

Data path (v3, bf16 compute / f32 accumulate):
  - Expert weights w13/w2, token activations, and combine weights are bf16
    (host-cast); every routed-path matmul runs at 1 cycle/row.  Router and
    shared expert run in f32r (also 1 cycle/row at free-size >= 256).
  - Token dispatch: one TRANSPOSED dma_gather per expert straight from the
    bf16 x table in DRAM -- the gather itself produces the [feature, slot]
    rhs layout stage 1 needs, eliminating all on-chip transposes.  Gathers
    are software-pipelined (gather j+1 issues before expert j's scatter).
  - Per-slot combine weights for all 8 experts come from four 512-index
    gathers of a duplicated-column cmb table (256B elems), applied as the
    per-partition `scale` of the stage-2 PSUM->SBUF copy.
  - All 8 experts' weights preload to SBUF upfront (sync/scalar queues
    alternating) and stay resident; scatter-add and the ReduceScatter stay
    f32 (bf16 CCE scatter/RS is numerically broken on HW, and DGE ops with
    >512 gather / >256 scatter indices hang or corrupt).
"""

import numpy as np
import ml_dtypes

import concourse.bass as bass
import concourse.bacc as bacc
import concourse.mybir as mybir
import concourse.tile as tile
from concourse.bass_utils import run_bass_kernel_spmd
from concourse.masks import make_identity

F32 = mybir.dt.float32
F32R = mybir.dt.float32r
BF16 = mybir.dt.bfloat16
I16 = mybir.dt.int16
U32 = mybir.dt.uint32
U8 = mybir.dt.uint8

# Model constants (hardcoded per contest rules)
E = 64          # experts
TOPK = 8
NG = 8          # groups
TOPKG = 4       # groups selected
SCALE = 2.5
H = 768         # hidden
I = 384         # routed expert intermediate
SI = 768        # shared expert intermediate
T = 1024        # tokens
NCORES = 8
EPC = E // NCORES     # experts per core = 8
SIPC = SI // NCORES   # shared-intermediate rows per core = 96
C = 256               # per-expert token capacity (avg load is 128)
TCH = T // 128        # token chunks = 8
HCH = H // 128        # hidden chunks = 6
ICH = I // 128        # intermediate chunks = 3
XELEM = 896           # bf16 elems per xcmb row: 768 x | 64 cmb | 64 cmb copy
XCH = H // 128        # gathered x chunks = 6
BIG = 1.0e30


def r32(ap):
    return ap.bitcast(F32R)


def build_nc(debug=False):
    nc = bacc.Bacc(num_devices=NCORES)

    # ---------------- I/O ----------------
    xT_d = nc.declare_dram_parameter("xT", [H, T], F32R, isOutput=False)
    xbf_d = nc.declare_dram_parameter("x_bf", [T + 1, H], BF16, isOutput=False)
    gwT_d = nc.declare_dram_parameter("gwT", [H, E], F32R, isOutput=False)
    eb_d = nc.declare_dram_parameter("ebias_b", [128, E], F32, isOutput=False)
    w13_d = nc.declare_dram_parameter("w13T", [EPC, H, 2 * I], BF16, isOutput=False)
    w2_d = nc.declare_dram_parameter("w2T", [EPC, I, H], BF16, isOutput=False)
    wsg_d = nc.declare_dram_parameter("wsgT", [H, SIPC], F32R, isOutput=False)
    wsu_d = nc.declare_dram_parameter("wsuT", [H, SIPC], F32R, isOutput=False)
    wsd_d = nc.declare_dram_parameter("wsdT", [SIPC, H], F32R, isOutput=False)
    tok_d = nc.declare_dram_parameter("tokid1", [T, 1], F32, isOutput=False)
    slot_d = nc.declare_dram_parameter("slotidx", [16, 16], F32, isOutput=False)
    out_d = nc.declare_dram_parameter("out", [T // NCORES, H], F32, isOutput=True)

    # ---------------- internal DRAM ----------------
    cmb2_d = nc.dram_tensor("cmb2_d", [T + 1, 2 * E], BF16)
    vals_d = nc.dram_tensor("vals_d", [EPC, T], F32)
    idx_d = nc.dram_tensor("idx_d", [16, 8 * 16], I16)
    acc_d = nc.dram_tensor("acc_d", [T + 1, H], F32)
    rs_d = nc.dram_tensor("rs_d", [T // NCORES, H], F32)

    with tile.TileContext(nc) as tc:
        with (
            tc.tile_pool(name="const", bufs=1) as constp,
            tc.tile_pool(name="wts", bufs=1) as wtsp,
            tc.tile_pool(name="xt", bufs=1) as xtp,
            tc.tile_pool(name="route", bufs=1) as routep,
            tc.tile_pool(name="small", bufs=2) as smallp,
            tc.tile_pool(name="work", bufs=2) as workp,
            tc.tile_pool(name="xg", bufs=1) as xgp,
            tc.tile_pool(name="scp", bufs=1) as scp,
            tc.tile_pool(name="psum", bufs=8, space="PSUM") as psp,
        ):
            # DMA issue engines, round-robined to spread queues
            dmae = [nc.sync, nc.scalar]

            # ---------------- constants / inputs ----------------
            ident = constp.tile([128, 128], F32, tag="ident")
            make_identity(nc, ident[:])
            ebias = constp.tile([128, E], F32, tag="ebias")
            nc.sync.dma_start(out=ebias[:], in_=eb_d[:, :])
            tokid1 = constp.tile([128, TCH], F32, tag="tokid1")
            nc.sync.dma_start(
                out=tokid1[:], in_=tok_d.ap().rearrange("(c p) o -> p (c o)", p=128)
            )
            slotidx = constp.tile([16, 16], F32, tag="slotidx")
            nc.sync.dma_start(out=slotidx[:], in_=slot_d[:, :])
            ones1 = constp.tile([128, 1], F32, tag="ones1")
            nc.vector.memset(ones1[:], 1.0)

            xT = xtp.tile([128, HCH, T], F32R, tag="xT")
            xTsrc = xT_d.ap().rearrange("(k p) t -> p k t", p=128)
            for k in range(HCH):
                eng = nc.scalar if k % 2 == 0 else nc.sync
                eng.dma_start(out=xT[:, k, :], in_=xTsrc[:, k : k + 1, :])
            gwT = constp.tile([128, HCH, E], F32R, tag="gwT")
            nc.scalar.dma_start(
                out=gwT[:], in_=gwT_d.ap().rearrange("(k p) e -> p k e", p=128)
            )
            wsg = constp.tile([128, HCH, SIPC], F32R, tag="wsg")
            nc.scalar.dma_start(
                out=wsg[:], in_=wsg_d.ap().rearrange("(k p) i -> p k i", p=128)
            )
            wsu = constp.tile([128, HCH, SIPC], F32R, tag="wsu")
            nc.scalar.dma_start(
                out=wsu[:], in_=wsu_d.ap().rearrange("(k p) i -> p k i", p=128)
            )
            wsd = constp.tile([SIPC, H], F32R, tag="wsd")
            nc.scalar.dma_start(out=wsd[:], in_=wsd_d[:, :])

            # zero both cmb column copies of the dummy row T
            zrow = smallp.tile([1, 2 * E], BF16, tag="zrow")
            nc.vector.memset(zrow[:], 0.0)
            nc.sync.dma_start(out=cmb2_d[T : T + 1, :], in_=zrow[:])

            # ------------- preload ALL expert weights (bf16) -------------
            w13s, w2s = [], []
            for j in range(EPC):
                w13 = wtsp.tile([128, HCH, 2 * I], BF16, tag=f"w13_{j}")
                dmae[j % 2].dma_start(
                    out=w13[:], in_=w13_d[j].rearrange("(k p) i -> p k i", p=128)
                )
                w2 = wtsp.tile([128, ICH, H], BF16, tag=f"w2_{j}")
                dmae[(j + 1) % 2].dma_start(
                    out=w2[:], in_=w2_d[j].rearrange("(k p) h -> p k h", p=128)
                )
                w13s.append(w13)
                w2s.append(w2)

            # ---------------- router: logitsT = gwT.T @ xT ----------------
            lgsb = routep.tile([64, T], F32, tag="lgsb")
            for n in range(2):
                lgp = psp.tile([64, 512], F32, tag="ps")
                for k in range(HCH):
                    nc.tensor.matmul(
                        out=lgp[:],
                        lhsT=gwT[:, k, :],
                        rhs=xT[:, k, n * 512 : (n + 1) * 512],
                        start=(k == 0),
                        stop=(k == HCH - 1),
                    )
                nc.vector.tensor_copy(out=lgsb[:, n * 512 : (n + 1) * 512], in_=lgp[:])

            # routing state, [128, chunk, expert] with experts on the free dim
            scores = routep.tile([128, TCH, E], F32, tag="scores")
            swb = routep.tile([128, TCH, E], F32, tag="swb")
            for c in range(TCH):
                lt = psp.tile([128, 64], F32, tag="ps")
                nc.tensor.transpose(
                    out=lt[:], in_=lgsb[:, c * 128 : (c + 1) * 128],
                    identity=ident[:64, :64],
                )
                nc.scalar.activation(
                    out=scores[:, c, :], in_=lt[:],
                    func=mybir.ActivationFunctionType.Sigmoid,
                )
            nc.vector.tensor_tensor(
                out=swb[:], in0=scores[:],
                in1=ebias[:, None, :].to_broadcast([128, TCH, E]),
                op=mybir.AluOpType.add,
            )

            # group scores = top1 + top2 per group, batched over chunks
            swb4 = swb[:].rearrange("p c (g e) -> p (c g) e", e=NG)
            m1 = routep.tile([128, TCH * NG], F32, tag="m1")
            nc.vector.tensor_reduce(
                out=m1[:], in_=swb4, axis=mybir.AxisListType.X,
                op=mybir.AluOpType.max,
            )
            eq = routep.tile([128, TCH * NG, NG], F32, tag="masked")
            nc.vector.tensor_tensor(
                out=eq[:], in0=swb4,
                in1=m1[:, :, None].to_broadcast([128, TCH * NG, NG]),
                op=mybir.AluOpType.is_equal,
            )
            nc.vector.tensor_scalar(
                out=eq[:], in0=eq[:], scalar1=-BIG, scalar2=None,
                op0=mybir.AluOpType.mult,
            )
            nc.vector.tensor_tensor(
                out=eq[:], in0=eq[:], in1=swb4, op=mybir.AluOpType.add
            )
            m2 = routep.tile([128, TCH * NG], F32, tag="m2")
            nc.vector.tensor_reduce(
                out=m2[:], in_=eq[:], axis=mybir.AxisListType.X,
                op=mybir.AluOpType.max,
            )
            gs = routep.tile([128, TCH, NG], F32, tag="gs")
            nc.vector.tensor_tensor(
                out=gs[:].rearrange("p c g -> p (c g)"), in0=m1[:], in1=m2[:],
                op=mybir.AluOpType.add,
            )
            # top-4 group mask, then masked score top-8 per token
            g4 = routep.tile([128, TCH, 8], F32, tag="g4")
            masked = routep.tile([128, TCH, E], F32, tag="masked")
            nmask = routep.tile([128, TCH, E], F32, tag="nmask")
            gmask = routep.tile([128, TCH * NG], F32, tag="gmask")
            for c in range(TCH):
                nc.vector.max(out=g4[:, c, :], in_=gs[:, c, :])
            for c in range(TCH):
                nc.vector.tensor_scalar(
                    out=gmask[:, c * NG : (c + 1) * NG], in0=gs[:, c, :],
                    scalar1=g4[:, c, TOPKG - 1 : TOPKG], scalar2=None,
                    op0=mybir.AluOpType.is_ge,
                )
            nc.vector.tensor_tensor(
                out=masked[:].rearrange("p c (g e) -> p (c g) e", e=NG),
                in0=swb4,
                in1=gmask[:, :, None].to_broadcast([128, TCH * NG, NG]),
                op=mybir.AluOpType.mult,
            )
            t8 = routep.tile([128, TCH, 8], F32, tag="t8")
            for c in range(TCH):
                nc.vector.max(out=t8[:, c, :], in_=masked[:, c, :])
            for c in range(TCH):
                nc.vector.tensor_scalar(
                    out=nmask[:, c, :], in0=masked[:, c, :],
                    scalar1=t8[:, c, TOPK - 1 : TOPK], scalar2=None,
                    op0=mybir.AluOpType.is_ge,
                )
            selp = routep.tile([128, TCH, E], F32, tag="swb")
            nc.vector.tensor_tensor(
                out=selp[:], in0=scores[:], in1=nmask[:], op=mybir.AluOpType.mult
            )
            den = routep.tile([128, TCH], F32, tag="den")
            nc.vector.tensor_reduce(
                out=den[:], in_=selp[:], axis=mybir.AxisListType.X,
                op=mybir.AluOpType.add,
            )
            nc.vector.tensor_scalar(
                out=den[:], in0=den[:], scalar1=1e-20, scalar2=None,
                op0=mybir.AluOpType.add,
            )
            rec = routep.tile([128, TCH], F32, tag="rec")
            nc.vector.reciprocal(out=rec[:], in_=den[:])
            nc.vector.tensor_scalar(
                out=rec[:], in0=rec[:], scalar1=SCALE, scalar2=None,
                op0=mybir.AluOpType.mult,
            )
            selbf = routep.tile([128, TCH, E], BF16, tag="selbf")
            nc.vector.tensor_tensor(
                out=selbf[:], in0=selp[:],
                in1=rec[:, :, None].to_broadcast([128, TCH, E]),
                op=mybir.AluOpType.mult,
            )
            # combine weights -> cmb columns of the gather table (two copies,
            # so the weight-gather elem is 256B-aligned)
            for c in range(TCH):
                nc.scalar.dma_start(
                    out=cmb2_d[c * 128 : (c + 1) * 128, 0:E],
                    in_=selbf[:, c, :],
                )
                nc.sync.dma_start(
                    out=cmb2_d[c * 128 : (c + 1) * 128, E : 2 * E],
                    in_=selbf[:, c, :],
                )

            # per-token markers for my experts -> counts + token-id lists
            valsb = routep.tile([128, TCH, EPC], F32, tag="valsb")
            valsT = routep.tile([EPC, T], F32, tag="valsT")
            cnt_ps = psp.tile([EPC, 1], F32, tag="ps")
            for c in range(TCH):
                nc.tensor.matmul(
                    out=cnt_ps[:], lhsT=nmask[:, c, 0:EPC], rhs=ones1[:],
                    start=(c == 0), stop=(c == TCH - 1),
                )
                nc.vector.tensor_scalar(
                    out=valsb[:, c, :], in0=nmask[:, c, 0:EPC],
                    scalar1=tokid1[:, c : c + 1], scalar2=None,
                    op0=mybir.AluOpType.mult,
                )
            nc.vector.tensor_scalar(
                out=valsb[:], in0=valsb[:], scalar1=-1.0, scalar2=None,
                op0=mybir.AluOpType.add,
            )
            for c in range(TCH):
                vt = psp.tile([EPC, 128], F32, tag="ps")
                nc.tensor.transpose(out=vt[:], in_=valsb[:, c, :], identity=ident[:])
                nc.vector.tensor_copy(
                    out=valsT[:, c * 128 : (c + 1) * 128], in_=vt[:]
                )

            # counts -> row layout on partition 0 (gpsimd needs base partition 0)
            cnt = routep.tile([EPC, 1], F32, tag="cnt")
            nc.vector.tensor_copy(out=cnt[:], in_=cnt_ps[:])
            cntrow_ps = psp.tile([1, EPC], F32, tag="ps")
            nc.tensor.transpose(
                out=cntrow_ps[:], in_=cnt[:], identity=ident[:EPC, :EPC]
            )
            cntrow = routep.tile([1, EPC], F32, tag="cntrow")
            nc.vector.tensor_copy(out=cntrow[:], in_=cntrow_ps[:])
            # broadcast count j to 16 partitions (for slot sanitation)
            cntb = routep.tile([16, EPC], F32, tag="cntb")
            for j in range(EPC):
                nc.gpsimd.partition_broadcast(
                    out_ap=cntb[:, j : j + 1], in_ap=cntrow[0:1, j : j + 1]
                )

            # valsT -> DRAM -> per-expert 16-partition-wrapped tiles [16, 64]
            nc.sync.dma_start(out=vals_d[:, :], in_=valsT[:])
            v16 = []
            for j in range(EPC):
                vt16 = routep.tile([16, T // 16], F32, tag=f"v16_{j}")
                nc.sync.dma_start(
                    out=vt16[:],
                    in_=vals_d[j].rearrange("(p f) -> p f", p=16),
                )
                v16.append(vt16)

            # compact per-expert token lists (sparse_gather) + sanitize tails
            padT = routep.tile([16, 16], F32, tag="padT")
            nc.vector.memset(padT[:], float(T))
            idxf = routep.tile([16, EPC * 16], F32, tag="idxf")
            nfound = routep.tile([1, EPC], U32, tag="nfound")
            idxs = routep.tile([16, EPC * 16], F32, tag="idxs")
            idx16 = routep.tile([16, EPC * 16], I16, tag="idx16")
            for j in range(EPC):
                nc.gpsimd.sparse_gather(
                    out=idxf[:, j * 16 : (j + 1) * 16],
                    in_=v16[j][:],
                    num_found=nfound[:, j : j + 1],
                )
                # sanitize: slots >= count[j] -> dummy row T (gathers zeros;
                # its combine weight row is 0 so the contribution is 0 and
                # the scatter-add target is the dummy row).
                keep = smallp.tile([16, 16], U8, tag="keep")
                nc.vector.tensor_scalar(
                    out=keep[:], in0=slotidx[:], scalar1=cntb[:, j : j + 1],
                    scalar2=None, op0=mybir.AluOpType.is_lt,
                )
                nc.vector.select(
                    out=idxs[:, j * 16 : (j + 1) * 16], mask=keep[:],
                    on_true=idxf[:, j * 16 : (j + 1) * 16], on_false=padT[:],
                )
            # replicate idx rows to all 8 16-partition groups (via DRAM
            # bounce, per expert so expert j's gather starts as soon as its
            # own compaction lands; step-0 AP repeats the 16 rows 8x)
            idxr = routep.tile([128, EPC * 16], I16, tag="idxr")
            for j in range(EPC):
                nc.vector.tensor_copy(
                    out=idx16[:, j * 16 : (j + 1) * 16],
                    in_=idxs[:, j * 16 : (j + 1) * 16],
                )
                eng = nc.sync if j % 2 == 0 else nc.scalar
                eng.dma_start(
                    out=idx_d[:, j * 16 : (j + 1) * 16],
                    in_=idx16[:, j * 16 : (j + 1) * 16],
                )
                eng.dma_start(
                    out=idxr[:, j * 16 : (j + 1) * 16],
                    in_=bass.AP(
                        idx_d, j * 16, [[0, 8], [EPC * 16, 16], [1, 16]]
                    ),
                )

            # ---------------- shared expert (TP slice) ----------------
            hsh = routep.tile([SIPC, T], F32R, tag="hsh")
            for n in range(2):
                hg = psp.tile([SIPC, 512], F32, tag="ps")
                hu = psp.tile([SIPC, 512], F32, tag="ps")
                for k in range(HCH):
                    nc.tensor.matmul(
                        out=hg[:], lhsT=wsg[:, k, :],
                        rhs=xT[:, k, n * 512 : (n + 1) * 512],
                        start=(k == 0), stop=(k == HCH - 1),
                    )
                for k in range(HCH):
                    nc.tensor.matmul(
                        out=hu[:], lhsT=wsu[:, k, :],
                        rhs=xT[:, k, n * 512 : (n + 1) * 512],
                        start=(k == 0), stop=(k == HCH - 1),
                    )
                hs_sl = hsh[:, n * 512 : (n + 1) * 512]
                nc.scalar.activation(
                    out=hs_sl, in_=hg[:], func=mybir.ActivationFunctionType.Sigmoid
                )
                nc.vector.tensor_tensor(
                    out=hs_sl, in0=hs_sl, in1=hg[:], op=mybir.AluOpType.mult
                )
                nc.vector.tensor_tensor(
                    out=hs_sl, in0=hs_sl, in1=hu[:], op=mybir.AluOpType.mult
                )
            for c in range(TCH):
                so = workp.tile([128, H], F32, tag="so")
                for n2 in range(2):
                    sp = psp.tile([128, 384], F32, tag="ps")
                    nc.tensor.matmul(
                        out=sp[:],
                        lhsT=hsh[:, c * 128 : (c + 1) * 128],
                        rhs=wsd[:, n2 * 384 : (n2 + 1) * 384],
                        start=True, stop=True,
                    )
                    nc.vector.tensor_copy(
                        out=so[:, n2 * 384 : (n2 + 1) * 384], in_=sp[:]
                    )
                nc.scalar.dma_start(
                    out=acc_d[c * 128 : (c + 1) * 128, :], in_=so[:]
                )

            # ---------------- routed experts ----------------
            # pre-issue the first two token gathers so expert-0 compute can
            # start before the combine-weight gathers occupy the DGE queue
            xcT_a = xgp.tile([128, XCH, C], BF16, tag="xcT0")
            xcT_b = xgp.tile([128, XCH, C], BF16, tag="xcT1")
            xcTs = [xcT_a, xcT_b]
            for jj in range(2):
                nc.gpsimd.dma_gather(
                    out_ap=xcTs[jj][:], in_ap=xbf_d[:, :],
                    idxs_ap=idxr[:, jj * 16 : (jj + 1) * 16],
                    num_idxs=C, num_idxs_reg=C, elem_size=H,
                    transpose=True,
                )
            # batched gathers for all experts' per-slot combine weights:
            # wgall[slot%128, 2j+slot//128, e] = cmb[token(slot), e]
            wgall = routep.tile([128, 2 * EPC, 2 * E], BF16, tag="wgall")
            for q in range(4):
                nc.gpsimd.dma_gather(
                    out_ap=wgall[:, 4 * q : 4 * (q + 1), :],
                    in_ap=cmb2_d[:, :],
                    idxs_ap=idxr[:, 32 * q : 32 * (q + 1)],
                    num_idxs=512, num_idxs_reg=512,
                    elem_size=2 * E,
                )
            wsc = routep.tile([128, 2 * EPC], F32, tag="wsc")
            for j in range(EPC):
                for ci in range(2):
                    nc.vector.tensor_copy(
                        out=wsc[:, 2 * j + ci : 2 * j + ci + 1],
                        in_=wgall[:, 2 * j + ci, j : j + 1],
                    )
            sc_pair = None
            for j in range(EPC):
                w13, w2 = w13s[j], w2s[j]
                xcT = xcTs[j % 2]

                # stage 1: h = silu(w1 @ x) * (w3 @ x)
                hj = workp.tile([128, ICH, C], BF16, tag="hj")
                for m in range(ICH):
                    h13 = psp.tile([128, 512], F32, tag="ps")
                    for k in range(HCH):
                        nc.tensor.matmul(
                            out=h13[:, 0:C],
                            lhsT=w13[:, k, m * 128 : (m + 1) * 128],
                            rhs=xcT[:, k, :],
                            start=(k == 0), stop=(k == HCH - 1),
                        )
                    for k in range(HCH):
                        nc.tensor.matmul(
                            out=h13[:, C : 2 * C],
                            lhsT=w13[:, k, I + m * 128 : I + (m + 1) * 128],
                            rhs=xcT[:, k, :],
                            start=(k == 0), stop=(k == HCH - 1),
                        )
                    hsil = workp.tile([128, C], F32, tag="hsil")
                    nc.scalar.activation(
                        out=hsil[:], in_=h13[:, 0:C],
                        func=mybir.ActivationFunctionType.Sigmoid,
                    )
                    nc.vector.tensor_tensor(
                        out=hsil[:], in0=hsil[:], in1=h13[:, 0:C],
                        op=mybir.AluOpType.mult,
                    )
                    nc.vector.tensor_tensor(
                        out=hj[:, m, :], in0=hsil[:], in1=h13[:, C : 2 * C],
                        op=mybir.AluOpType.mult,
                    )

                # prefetch expert j+2's tokens while stage 2 runs
                if j + 2 < EPC:
                    nc.gpsimd.dma_gather(
                        out_ap=xcTs[j % 2][:], in_ap=xbf_d[:, :],
                        idxs_ap=idxr[:, (j + 2) * 16 : (j + 3) * 16],
                        num_idxs=C, num_idxs_reg=C, elem_size=H,
                        transpose=True,
                    )

                # stage 2: out2 = (h @ w2T) * combine_weight  (out: [slot, H])
                if j % 2 == 0:
                    sc_pair = scp.tile([128, 4, H], F32, tag="sc")
                for ci in range(C // 128):
                    psA = psp.tile([128, 384], F32, tag="ps")
                    psB = psp.tile([128, 384], F32, tag="ps")
                    for k in range(ICH):
                        nc.tensor.matmul(
                            out=psA[:],
                            lhsT=hj[:, k, ci * 128 : (ci + 1) * 128],
                            rhs=w2[:, k, 0:384],
                            start=(k == 0), stop=(k == ICH - 1),
                        )
                        nc.tensor.matmul(
                            out=psB[:],
                            lhsT=hj[:, k, ci * 128 : (ci + 1) * 128],
                            rhs=w2[:, k, 384:768],
                            start=(k == 0), stop=(k == ICH - 1),
                        )
                    wcol = wsc[:, 2 * j + ci : 2 * j + ci + 1]
                    sl = 2 * (j % 2) + ci
                    nc.vector.tensor_scalar(
                        out=sc_pair[:, sl, 0:384], in0=psA[:], scalar1=wcol,
                        scalar2=None, op0=mybir.AluOpType.mult,
                    )
                    nc.scalar.activation(
                        out=sc_pair[:, sl, 384:768], in_=psB[:],
                        func=mybir.ActivationFunctionType.Copy, scale=wcol,
                    )

                if j % 2 == 1:
                    nc.gpsimd.dma_scatter_add(
                        out_ap=acc_d[:, :], in_ap=sc_pair[:, 0:2, :],
                        idxs_ap=idxr[:, (j - 1) * 16 : j * 16],
                        num_idxs=C, num_idxs_reg=C, elem_size=H,
                    )
                    nc.gpsimd.dma_scatter_add(
                        out_ap=acc_d[:, :], in_ap=sc_pair[:, 2:4, :],
                        idxs_ap=idxr[:, j * 16 : (j + 1) * 16],
                        num_idxs=C, num_idxs_reg=C, elem_size=H,
                    )

            # ---------------- cross-core reduce ----------------
            nc.gpsimd.collective_compute(
                "ReduceScatter",
                mybir.AluOpType.add,
                replica_groups=[list(range(NCORES))],
                ins=[acc_d[0:T, :]],
                outs=[rs_d[:, :]],
            )
            nc.sync.dma_start(out=out_d[:, :], in_=rs_d[:, :])

    return nc


def make_core_inputs(inputs):
    """Host-side sharding: returns the per-core input maps (list of dicts)."""
    x = np.ascontiguousarray(np.asarray(inputs["hidden_states"], np.float32))
    gate_w = np.asarray(inputs["gate_w"], np.float32)
    e_bias = np.asarray(inputs["e_bias"], np.float32)
    w1 = np.asarray(inputs["w1"], np.float32)
    w3 = np.asarray(inputs["w3"], np.float32)
    w2 = np.asarray(inputs["w2"], np.float32)
    ws_gate = np.asarray(inputs["ws_gate"], np.float32)
    ws_up = np.asarray(inputs["ws_up"], np.float32)
    ws_down = np.asarray(inputs["ws_down"], np.float32)

    bf = ml_dtypes.bfloat16
    xT = np.ascontiguousarray(x.T)
    x_bf = np.zeros((T + 1, H), bf)
    x_bf[:T] = x.astype(bf)
    # [E, H, 2I] bf16: w1.T | w3.T per expert
    w13_all = np.empty((E, H, 2 * I), bf)
    w13_all[:, :, :I] = np.transpose(w1, (0, 2, 1)).astype(bf)
    w13_all[:, :, I:] = np.transpose(w3, (0, 2, 1)).astype(bf)
    w2_all = np.ascontiguousarray(np.transpose(w2, (0, 2, 1)).astype(bf))

    tokid1 = (np.arange(T, dtype=np.float32) + 1.0).reshape(T, 1)
    slotidx = (
        np.arange(16, dtype=np.float32)[:, None]
        + 16.0 * np.arange(16, dtype=np.float32)[None, :]
    )  # slot(p, f) = f*16 + p
    maps = []
    for r in range(NCORES):
        rot = np.roll(np.arange(E), -EPC * r)
        mine = rot[:EPC]
        sl = slice(r * SIPC, (r + 1) * SIPC)
        maps.append(
            {
                "xT": xT,
                "x_bf": x_bf,
                "gwT": np.ascontiguousarray(gate_w[rot].T),
                "ebias_b": np.broadcast_to(e_bias[rot], (128, E)).copy(),
                "w13T": np.ascontiguousarray(w13_all[mine]),
                "w2T": np.ascontiguousarray(w2_all[mine]),
                "wsgT": np.ascontiguousarray(ws_gate[sl].T),
                "wsuT": np.ascontiguousarray(ws_up[sl].T),
                "wsdT": np.ascontiguousarray(ws_down[:, sl].T),
                "tokid1": tokid1,
                "slotidx": slotidx,
            }
        )
    return maps


_NC_CACHE = None


def kernel(**inputs) -> np.ndarray:
    global _NC_CACHE
    if _NC_CACHE is None:
        nc = build_nc()
        nc.finalize()
        _NC_CACHE = nc
    nc = _NC_CACHE
    in_maps = make_core_inputs(inputs)
    res = run_bass_kernel_spmd(nc, in_maps, list(range(NCORES)))
    out = np.concatenate([res.results[i]["out"] for i in range(NCORES)], axis=0)
    return out.astype(np.float32)
